# revision 9
# baseline (speedup 1.0000x reference)
"""Trainium2 Bass kernel for nn_DQN_57904749085018 (gnn_message_passing).

Computation (reference semantics):
    g   = x[:, idx]                                  [B, S, L] gather
    h   = (g - mean) * rsqrt(var+eps) * gamma + beta [B, S, L] batchnorm (eval)
    h1  = tanh(einsum('bsl,sol->bso', h, W1) + b1)   [B, S, 3]
    h2  = tanh(einsum('bsk,sok->bso', h1, W2) + b2)  [B, S, 2]
    a, sb = h2[..., 0], h2[..., 1]
    out[b,i,j] = tanh(a[b,i]*W3[i,j,0] + sb[b,j]*W3[i,j,1] + b3[i,j])
    -> reshape [B, S*S]

Kernel strategy (pure data parallel over 8 cores, batch-sharded), fp16
end-to-end (measured rel err ~5e-4 against the fp64 oracle, tolerance 2e-2):
  * gather + batchnorm + Linear1 fold into one dense matmul x @ Weff.T;
    x is padded to 512 features with a ones column at 407 so the biases
    ride along as ordinary weight rows.  x arrives transposed in SBUF via
    the HWDGE xbar DMA-transpose (no PE transposes).
  * the pairwise head out[b,(i,j)] = tanh(a_i w0_ij + sb_j w1_ij + b_ij)
    runs as two PSUM-accumulated matmuls per 512-col window:
      - a-part: K=100 one-hot rows (nonzero iff k==i),
      - sb-part: K=101 (100 servers, nonzero iff k==j, + a ones row
        carrying b3).
    fp16 tables shrink the pairwise read from 8.2 MB f32 to 4.1 MB.
  * output is written as fp16 (half the HBM write traffic) in one
    contiguous 2.56 MB DMA per 128-row block, alternating sync/gpsimd
    rings; the host upcasts to fp32.
"""

import sys

import numpy as np

if "/opt/trn_rl_repo" not in sys.path:
    sys.path.insert(0, "/opt/trn_rl_repo")

import concourse.bacc as bacc
import concourse.mybir as mybir
from concourse import bass_utils
from concourse.tile import TileContext

S = 100
L = 13
FEAT = 4 * S + 7  # 407
FP = 512  # padded feature width (col 407 = ones, 408.. = zero)
B = 8192
EPS = 1e-5
N_CORES = 8
BL = B // N_CORES  # 1024 batch rows per core
SS = S * S  # 10000
WIN = 512  # output column window (one PSUM bank)
NWIN = (SS + WIN - 1) // WIN  # 20 (19 full + 272 tail)
F16 = mybir.dt.float16
F32 = mybir.dt.float32

# wt pack layout (columns of the [128, 1800] fp16 tile)
WT_WEFF = [0, 300, 600, 900]  # WeffT feature-chunk k at col 300k, width 300
WT_W2 = [1200, 1400, 1600]  # W2effT k-chunk c, width 200
WT_COLS = 1800

_module_cache = None


def _build_indices():
    idx = [[2 * i, 2 * i + 1] for i in range(S)]
    start = 2 * S
    for k in range(S):
        u, v = k, (k + 1) % S
        idx[u].extend([start, start + 1])
        idx[v].extend([start, start + 1])
        start += 2
    g0 = 4 * S
    for i in range(S):
        idx[i].extend(range(g0, g0 + 7))
    return np.asarray(idx, dtype=np.int64)


def _host_weights(inputs):
    f64 = np.float64
    gamma = np.asarray(inputs["gamma"], f64)
    beta = np.asarray(inputs["beta"], f64)
    mean = np.asarray(inputs["mean"], f64)
    var = np.asarray(inputs["var"], f64)
    W1 = np.asarray(inputs["W1"], f64)  # [S, 3, L]
    b1 = np.asarray(inputs["b1"], f64)  # [S, 3]
    W2 = np.asarray(inputs["W2"], f64)  # [S, 2, 3]
    b2 = np.asarray(inputs["b2"], f64)  # [S, 2]
    W3 = np.asarray(inputs["W3"], f64)  # [S, S, 2]
    b3 = np.asarray(inputs["b3"], f64)  # [S, S]
    idx = np.asarray(inputs["idx"], np.int64)  # [S, L]

    scale = gamma / np.sqrt(var + EPS)  # [S, L]
    shift = beta - mean * scale  # [S, L]

    # Weff[(s,o), f] = sum_l [idx[s,l]==f] W1[s,o,l]*scale[s,l]
    Wsc = W1 * scale[:, None, :]  # [S, 3, L]
    Weff = np.zeros((S, 3, FEAT), f64)
    s_ix = np.repeat(np.arange(S), 3 * L)
    o_ix = np.tile(np.repeat(np.arange(3), L), S)
    f_ix = np.repeat(idx[:, None, :], 3, axis=1).ravel()
    np.add.at(Weff, (s_ix, o_ix, f_ix), Wsc.ravel())
    Weff = Weff.reshape(3 * S, FEAT)
    beff = (b1 + np.einsum("sol,sl->so", W1, shift)).reshape(3 * S)

    # W2eff[(o2*S+s), (s*3+k)] = W2[s, o2, k]
    W2eff = np.zeros((2 * S, 3 * S), f64)
    for s in range(S):
        for o2 in range(2):
            W2eff[o2 * S + s, s * 3 : s * 3 + 3] = W2[s, o2, :]
    b2eff = b2.T.reshape(2 * S)

    wt = np.zeros((128, WT_COLS), np.float16)
    WeffT = Weff.T  # [FEAT, 300]
    for k, c in enumerate(WT_WEFF):
        f0 = 128 * k
        fw = min(128, FEAT - f0)
        wt[0:fw, c : c + 300] = WeffT[f0 : f0 + fw, :]
    wt[407 - 384, WT_WEFF[3] : WT_WEFF[3] + 300] = beff  # ones col at x[:,407]
    W2effT = W2eff.T  # [300, 200]
    for c_i, c in enumerate(WT_W2):
        wt[0:100, c : c + 200] = W2effT[c_i * 100 : (c_i + 1) * 100, :]
    wt[100, WT_W2[0] : WT_W2[0] + 200] = b2eff  # ones row of h1_0 carries b2

    # mt0[i, i*S+j] = W3[i,j,0] (one-hot in i)
    # mt1 rows 0..99: [j, i*S+j] = W3[i,j,1]; row 100 = b3 (vs asb ones row)
    mt0 = np.zeros((S, SS), np.float16)
    mt1 = np.zeros((S + 1, SS), np.float16)
    g = np.arange(SS)
    i_g = g // S
    j_g = g % S
    mt0[i_g, g] = W3[i_g, j_g, 0].astype(np.float16)
    mt1[j_g, g] = W3[i_g, j_g, 1].astype(np.float16)
    mt1[S, :] = b3.ravel()

    return {
        "wt": wt,
        "mt0": mt0,
        "mt1": mt1,
        "onesd": np.ones((1, 2048), np.float16),
    }


def _build_module():
    global _module_cache
    if _module_cache is not None:
        return _module_cache

    nc = bacc.Bacc("TRN2", target_bir_lowering=False, debug=False, num_devices=N_CORES)
    xin = nc.dram_tensor("xin", [BL, FP], F16, kind="ExternalInput").ap()
    wt_d = nc.dram_tensor("wt", [128, WT_COLS], F16, kind="ExternalInput").ap()
    mt0_d = nc.dram_tensor("mt0", [S, SS], F16, kind="ExternalInput").ap()
    mt1_d = nc.dram_tensor("mt1", [S + 1, SS], F16, kind="ExternalInput").ap()
    onesd = nc.dram_tensor("onesd", [1, 2048], F16, kind="ExternalInput").ap()
    yout = nc.dram_tensor("yout", [BL, SS], F16, kind="ExternalOutput").ap()

    TANH = mybir.ActivationFunctionType.Tanh

    with TileContext(nc) as tc:
        with (
            tc.tile_pool(name="const", bufs=1) as const,
            tc.tile_pool(name="ot_pool", bufs=2) as ot_pool,
            tc.tile_pool(name="ps_pool", bufs=2, space="PSUM") as ps_pool,
        ):
            # ---- loads ----
            wt_t = const.tile([128, WT_COLS], F16)
            nc.scalar.dma_start(wt_t[:], wt_d[:, :])
            xT = []
            for k in range(4):
                xt = const.tile([128, BL], F16, name=f"xT_{k}", tag=f"xT{k}")
                nc.sync.dma_start(
                    xt[:], xin[:, 128 * k : 128 * (k + 1)], transpose=True
                )
                xT.append(xt)
            mt0_t = const.tile([S, SS], F16)
            mt1_t = const.tile([S + 1, SS], F16)
            for e0 in range(0, SS, SS // 2):
                nc.scalar.dma_start(
                    mt0_t[:, e0 : e0 + SS // 2], mt0_d[:, e0 : e0 + SS // 2]
                )
                nc.scalar.dma_start(
                    mt1_t[:, e0 : e0 + SS // 2], mt1_d[:, e0 : e0 + SS // 2]
                )

            warm = const.tile([1, 8], F32)
            nc.scalar.activation(warm[:], wt_t[0:1, 0:8], TANH)  # tanh table preload

            h1 = []
            for m in range(3):
                rows = 101 if m == 0 else 100
                t = const.tile([rows, BL], F16, name=f"h1_{m}", tag=f"h1{m}")
                h1.append(t)
            asb = const.tile([S + 1, 2 * BL], F16)
            nc.gpsimd.dma_start(h1[0][100:101, :], onesd[0:1, 0:BL])
            nc.gpsimd.dma_start(asb[S : S + 1, :], onesd[0:1, 0 : 2 * BL])

            # ---- front: h1 = tanh(x @ Weff.T + beff) ----
            for m in range(3):
                pm = ps_pool.tile([128, 2048], F32, name="pm", tag="ps")
                for h in range(2):
                    dst = pm[0:100, h * 512 : (h + 1) * 512]
                    for k in range(4):
                        nc.tensor.matmul(
                            dst,
                            wt_t[0:128, WT_WEFF[k] + 100 * m : WT_WEFF[k] + 100 * (m + 1)],
                            xT[k][0:128, h * 512 : (h + 1) * 512],
                            start=(k == 0),
                            stop=(k == 3),
                        )
                nc.scalar.activation(h1[m][0:100, :], pm[0:100, 0:BL], TANH)

            # ---- front: a/sb = tanh(h1 @ W2eff.T + b2eff) ----
            for g in range(2):
                pm2 = ps_pool.tile([128, 2048], F32, name="pm2", tag="ps")
                for h in range(2):
                    dst = pm2[0:100, h * 512 : (h + 1) * 512]
                    for c in range(3):
                        kr = 101 if c == 0 else 100
                        nc.tensor.matmul(
                            dst,
                            wt_t[0:kr, WT_W2[c] + 100 * g : WT_W2[c] + 100 * (g + 1)],
                            h1[c][0:kr, h * 512 : (h + 1) * 512],
                            start=(c == 0),
                            stop=(c == 2),
                        )
                nc.scalar.activation(
                    asb[0:S, g * BL : (g + 1) * BL], pm2[0:100, 0:BL], TANH
                )

            # ---- final: out = tanh(a_i w0 + sb_j w1 + b3) ----
            for bs in range(BL // 128):
                ot = ot_pool.tile([128, SS], F16, name="ot", tag="ot")
                ca = bs * 128
                cb = BL + bs * 128
                for q in range(5):
                    qw = min(2048, SS - q * 2048)
                    pf = ps_pool.tile([128, 2048], F32, name="pf", tag="ps")
                    for wl in range(4):
                        w = q * 4 + wl
                        c0 = WIN * w
                        cw = min(WIN, SS - c0)
                        off = 512 * wl
                        nc.tensor.matmul(
                            pf[0:128, off : off + cw],
                            asb[0:S, ca : ca + 128],
                            mt0_t[0:S, c0 : c0 + cw],
                            start=True,
                            stop=False,
                        )
                        nc.tensor.matmul(
                            pf[0:128, off : off + cw],
                            asb[0 : S + 1, cb : cb + 128],
                            mt1_t[0 : S + 1, c0 : c0 + cw],
                            start=False,
                            stop=True,
                        )
                    nc.scalar.activation(
                        ot[:, q * 2048 : q * 2048 + qw], pf[0:128, 0:qw], TANH
                    )
                oeng = nc.sync if bs % 2 == 0 else nc.gpsimd
                oeng.dma_start(yout[bs * 128 : (bs + 1) * 128, :], ot[:, 0:SS])

    nc.compile()
    _module_cache = nc
    return nc


def _run(inputs, trace=False, trace_cores=None):
    nc = _build_module()
    hw = _host_weights(inputs)
    x = np.asarray(inputs["x"], np.float32)
    xpad = np.zeros((B, FP), np.float16)
    xpad[:, :FEAT] = x.astype(np.float16)
    xpad[:, FEAT] = 1.0
    in_maps = []
    for c in range(N_CORES):
        m = dict(hw)
        m["xin"] = xpad[c * BL : (c + 1) * BL]
        in_maps.append(m)
    kwargs = {}
    if trace:
        bass_utils.upload_artifacts = lambda tmpdir: tmpdir  # no cloud store here
        kwargs = dict(trace=True, trace_cores=trace_cores or [0])
    res = bass_utils.run_bass_kernel_spmd(
        nc, in_maps, core_ids=list(range(N_CORES)), **kwargs
    )
    out = np.concatenate(
        [res.results[c]["yout"] for c in range(N_CORES)], axis=0
    ).astype(np.float32)
    return out, res


def kernel(**inputs) -> np.ndarray:
    out, _ = _run(inputs)
    return out


# revision 11
# speedup vs baseline: 1.2180x; 1.2180x over previous
"""Trainium2 Bass kernel for nn_DQN_57904749085018 (gnn_message_passing).

Computation (reference semantics):
    g   = x[:, idx]                                  [B, S, L] gather
    h   = (g - mean) * rsqrt(var+eps) * gamma + beta [B, S, L] batchnorm (eval)
    h1  = tanh(einsum('bsl,sol->bso', h, W1) + b1)   [B, S, 3]
    h2  = tanh(einsum('bsk,sok->bso', h1, W2) + b2)  [B, S, 2]
    a, sb = h2[..., 0], h2[..., 1]
    out[b,i,j] = tanh(a[b,i]*W3[i,j,0] + sb[b,j]*W3[i,j,1] + b3[i,j])
    -> reshape [B, S*S]

Kernel strategy (pure data parallel over 8 cores, batch-sharded), fp16
end-to-end (measured rel err ~5e-4 against the fp64 oracle, tolerance 2e-2):
  * gather + batchnorm + Linear1 fold into one dense matmul x @ Weff.T;
    x is padded to 512 features with a ones column at 407 so the biases
    ride along as ordinary weight rows.  x arrives transposed in SBUF via
    the HWDGE xbar DMA-transpose (no PE transposes).
  * the pairwise head out[b,(i,j)] = tanh(a_i w0_ij + sb_j w1_ij + b_ij)
    runs as two PSUM-accumulated matmuls per 512-col window:
      - a-part: K=100 one-hot rows (nonzero iff k==i),
      - sb-part: K=101 (100 servers, nonzero iff k==j, + a ones row
        carrying b3).
    fp16 tables shrink the pairwise read from 8.2 MB f32 to 4.1 MB.
  * output is written as fp16 (half the HBM write traffic) in one
    contiguous 2.56 MB DMA per 128-row block, alternating sync/gpsimd
    rings; the host upcasts to fp32.
"""

import sys

import numpy as np

if "/opt/trn_rl_repo" not in sys.path:
    sys.path.insert(0, "/opt/trn_rl_repo")

import concourse.bacc as bacc
import concourse.mybir as mybir
from concourse import bass_utils
from concourse.tile import TileContext

S = 100
L = 13
FEAT = 4 * S + 7  # 407
FP = 512  # padded feature width (col 407 = ones, 408.. = zero)
B = 8192
EPS = 1e-5
N_CORES = 8
BL = B // N_CORES  # 1024 batch rows per core
SS = S * S  # 10000
WIN = 512  # output column window (one PSUM bank)
NWIN = (SS + WIN - 1) // WIN  # 20 (19 full + 272 tail)
F16 = mybir.dt.float16
F32 = mybir.dt.float32

# wt pack layout (columns of the [128, 1800] fp16 tile)
WT_WEFF = [0, 300, 600, 900]  # WeffT feature-chunk k at col 300k, width 300
WT_W2 = [1200, 1400, 1600]  # W2effT k-chunk c, width 200
WT_COLS = 1800

_module_cache = None


def _build_indices():
    idx = [[2 * i, 2 * i + 1] for i in range(S)]
    start = 2 * S
    for k in range(S):
        u, v = k, (k + 1) % S
        idx[u].extend([start, start + 1])
        idx[v].extend([start, start + 1])
        start += 2
    g0 = 4 * S
    for i in range(S):
        idx[i].extend(range(g0, g0 + 7))
    return np.asarray(idx, dtype=np.int64)


def _host_weights(inputs):
    f64 = np.float64
    gamma = np.asarray(inputs["gamma"], f64)
    beta = np.asarray(inputs["beta"], f64)
    mean = np.asarray(inputs["mean"], f64)
    var = np.asarray(inputs["var"], f64)
    W1 = np.asarray(inputs["W1"], f64)  # [S, 3, L]
    b1 = np.asarray(inputs["b1"], f64)  # [S, 3]
    W2 = np.asarray(inputs["W2"], f64)  # [S, 2, 3]
    b2 = np.asarray(inputs["b2"], f64)  # [S, 2]
    W3 = np.asarray(inputs["W3"], f64)  # [S, S, 2]
    b3 = np.asarray(inputs["b3"], f64)  # [S, S]
    idx = np.asarray(inputs["idx"], np.int64)  # [S, L]

    scale = gamma / np.sqrt(var + EPS)  # [S, L]
    shift = beta - mean * scale  # [S, L]

    # Weff[(s,o), f] = sum_l [idx[s,l]==f] W1[s,o,l]*scale[s,l]
    Wsc = W1 * scale[:, None, :]  # [S, 3, L]
    Weff = np.zeros((S, 3, FEAT), f64)
    s_ix = np.repeat(np.arange(S), 3 * L)
    o_ix = np.tile(np.repeat(np.arange(3), L), S)
    f_ix = np.repeat(idx[:, None, :], 3, axis=1).ravel()
    np.add.at(Weff, (s_ix, o_ix, f_ix), Wsc.ravel())
    Weff = Weff.reshape(3 * S, FEAT)
    beff = (b1 + np.einsum("sol,sl->so", W1, shift)).reshape(3 * S)

    # W2eff[(o2*S+s), (s*3+k)] = W2[s, o2, k]
    W2eff = np.zeros((2 * S, 3 * S), f64)
    for s in range(S):
        for o2 in range(2):
            W2eff[o2 * S + s, s * 3 : s * 3 + 3] = W2[s, o2, :]
    b2eff = b2.T.reshape(2 * S)

    wt = np.zeros((128, WT_COLS), np.float16)
    WeffT = Weff.T  # [FEAT, 300]
    for k, c in enumerate(WT_WEFF):
        f0 = 128 * k
        fw = min(128, FEAT - f0)
        wt[0:fw, c : c + 300] = WeffT[f0 : f0 + fw, :]
    wt[407 - 384, WT_WEFF[3] : WT_WEFF[3] + 300] = beff  # ones col at x[:,407]
    W2effT = W2eff.T  # [300, 200]
    for c_i, c in enumerate(WT_W2):
        wt[0:100, c : c + 200] = W2effT[c_i * 100 : (c_i + 1) * 100, :]
    wt[100, WT_W2[0] : WT_W2[0] + 200] = b2eff  # ones row of h1_0 carries b2

    # mt0[i, i*S+j] = W3[i,j,0] (one-hot in i)
    # mt1 rows 0..99: [j, i*S+j] = W3[i,j,1]; row 100 = b3 (vs asb ones row)
    mt0 = np.zeros((S, SS), np.float16)
    mt1 = np.zeros((S + 1, SS), np.float16)
    g = np.arange(SS)
    i_g = g // S
    j_g = g % S
    mt0[i_g, g] = W3[i_g, j_g, 0].astype(np.float16)
    mt1[j_g, g] = W3[i_g, j_g, 1].astype(np.float16)
    mt1[S, :] = b3.ravel()

    return {
        "wt": wt,
        "mt0": mt0,
        "mt1": mt1,
        "onesd": np.ones((1, 2048), np.float16),
    }


def _build_module():
    global _module_cache
    if _module_cache is not None:
        return _module_cache

    nc = bacc.Bacc("TRN2", target_bir_lowering=False, debug=False, num_devices=N_CORES)
    xin = nc.dram_tensor("xin", [BL, FP], F16, kind="ExternalInput").ap()
    wt_d = nc.dram_tensor("wt", [128, WT_COLS], F16, kind="ExternalInput").ap()
    mt0_d = nc.dram_tensor("mt0", [S, SS], F16, kind="ExternalInput").ap()
    mt1_d = nc.dram_tensor("mt1", [S + 1, SS], F16, kind="ExternalInput").ap()
    onesd = nc.dram_tensor("onesd", [1, 2048], F16, kind="ExternalInput").ap()
    yout = nc.dram_tensor("yout", [BL, SS], F16, kind="ExternalOutput").ap()

    TANH = mybir.ActivationFunctionType.Tanh

    with TileContext(nc) as tc:
        with (
            tc.tile_pool(name="const", bufs=1) as const,
            tc.tile_pool(name="ot_pool", bufs=2) as ot_pool,
            tc.tile_pool(name="ps_pool", bufs=2, space="PSUM") as ps_pool,
        ):
            # ---- loads ----
            # All bulk loads go through gpsimd (SWDGE): its descriptor
            # generation spreads partition lines across all 16 SDMA engines,
            # while HWDGE-dynamic loads clump onto one engine (~27 GiB/s).
            wt_t = const.tile([128, WT_COLS], F16)
            nc.gpsimd.dma_start(wt_t[:], wt_d[:, :])
            xT = []
            for k in range(4):
                xt = const.tile([128, BL], F16, name=f"xT_{k}", tag=f"xT{k}")
                nc.sync.dma_start(
                    xt[:], xin[:, 128 * k : 128 * (k + 1)], transpose=True
                )
                xT.append(xt)

            h1 = []
            for m in range(3):
                rows = 101 if m == 0 else 100
                t = const.tile([rows, BL], F16, name=f"h1_{m}", tag=f"h1{m}")
                h1.append(t)
            asb = const.tile([S + 1, 2 * BL], F16)
            nc.gpsimd.dma_start(h1[0][100:101, :], onesd[0:1, 0:BL])
            nc.gpsimd.dma_start(asb[S : S + 1, :], onesd[0:1, 0 : 2 * BL])

            mt0_t = const.tile([S, SS], F16)
            mt1_t = const.tile([S + 1, SS], F16)
            for e0 in range(0, SS, SS // 2):
                nc.gpsimd.dma_start(
                    mt0_t[:, e0 : e0 + SS // 2], mt0_d[:, e0 : e0 + SS // 2]
                )
                nc.gpsimd.dma_start(
                    mt1_t[:, e0 : e0 + SS // 2], mt1_d[:, e0 : e0 + SS // 2]
                )

            warm = const.tile([1, 8], F32)
            nc.scalar.activation(warm[:], wt_t[0:1, 0:8], TANH)  # tanh table preload

            # ---- front: h1 = tanh(x @ Weff.T + beff) ----
            for m in range(3):
                pm = ps_pool.tile([128, 2048], F32, name="pm", tag="ps")
                for h in range(2):
                    dst = pm[0:100, h * 512 : (h + 1) * 512]
                    for k in range(4):
                        nc.tensor.matmul(
                            dst,
                            wt_t[0:128, WT_WEFF[k] + 100 * m : WT_WEFF[k] + 100 * (m + 1)],
                            xT[k][0:128, h * 512 : (h + 1) * 512],
                            start=(k == 0),
                            stop=(k == 3),
                        )
                nc.scalar.activation(h1[m][0:100, :], pm[0:100, 0:BL], TANH)

            # ---- front: a/sb = tanh(h1 @ W2eff.T + b2eff) ----
            for g in range(2):
                pm2 = ps_pool.tile([128, 2048], F32, name="pm2", tag="ps")
                for h in range(2):
                    dst = pm2[0:100, h * 512 : (h + 1) * 512]
                    for c in range(3):
                        kr = 101 if c == 0 else 100
                        nc.tensor.matmul(
                            dst,
                            wt_t[0:kr, WT_W2[c] + 100 * g : WT_W2[c] + 100 * (g + 1)],
                            h1[c][0:kr, h * 512 : (h + 1) * 512],
                            start=(c == 0),
                            stop=(c == 2),
                        )
                nc.scalar.activation(
                    asb[0:S, g * BL : (g + 1) * BL], pm2[0:100, 0:BL], TANH
                )

            # ---- final: out = tanh(a_i w0 + sb_j w1 + b3) ----
            for bs in range(BL // 128):
                ot = ot_pool.tile([128, SS], F16, name="ot", tag="ot")
                ca = bs * 128
                cb = BL + bs * 128
                for q in range(5):
                    qw = min(2048, SS - q * 2048)
                    pf = ps_pool.tile([128, 2048], F32, name="pf", tag="ps")
                    for wl in range(4):
                        w = q * 4 + wl
                        c0 = WIN * w
                        cw = min(WIN, SS - c0)
                        off = 512 * wl
                        nc.tensor.matmul(
                            pf[0:128, off : off + cw],
                            asb[0:S, ca : ca + 128],
                            mt0_t[0:S, c0 : c0 + cw],
                            start=True,
                            stop=False,
                        )
                        nc.tensor.matmul(
                            pf[0:128, off : off + cw],
                            asb[0 : S + 1, cb : cb + 128],
                            mt1_t[0 : S + 1, c0 : c0 + cw],
                            start=False,
                            stop=True,
                        )
                    nc.scalar.activation(
                        ot[:, q * 2048 : q * 2048 + qw], pf[0:128, 0:qw], TANH
                    )
                    if q == 2:  # first half of the block is tanh'd — drain it
                        nc.sync.dma_start(
                            yout[bs * 128 : (bs + 1) * 128, 0 : SS // 2],
                            ot[:, 0 : SS // 2],
                        )
                nc.scalar.dma_start(
                    yout[bs * 128 : (bs + 1) * 128, SS // 2 : SS],
                    ot[:, SS // 2 : SS],
                )

    nc.compile()
    _module_cache = nc
    return nc


def _run(inputs, trace=False, trace_cores=None):
    nc = _build_module()
    hw = _host_weights(inputs)
    x = np.asarray(inputs["x"], np.float32)
    xpad = np.zeros((B, FP), np.float16)
    xpad[:, :FEAT] = x.astype(np.float16)
    xpad[:, FEAT] = 1.0
    in_maps = []
    for c in range(N_CORES):
        m = dict(hw)
        m["xin"] = xpad[c * BL : (c + 1) * BL]
        in_maps.append(m)
    kwargs = {}
    if trace:
        bass_utils.upload_artifacts = lambda tmpdir: tmpdir  # no cloud store here
        kwargs = dict(trace=True, trace_cores=trace_cores or [0])
    res = bass_utils.run_bass_kernel_spmd(
        nc, in_maps, core_ids=list(range(N_CORES)), **kwargs
    )
    out = np.concatenate(
        [res.results[c]["yout"] for c in range(N_CORES)], axis=0
    ).astype(np.float32)
    return out, res


def kernel(**inputs) -> np.ndarray:
    out, _ = _run(inputs)
    return out


# revision 15
# speedup vs baseline: 1.3486x; 1.1072x over previous
"""Trainium2 Bass kernel for nn_DQN_57904749085018 (gnn_message_passing).

Computation (reference semantics):
    g   = x[:, idx]                                  [B, S, L] gather
    h   = (g - mean) * rsqrt(var+eps) * gamma + beta [B, S, L] batchnorm (eval)
    h1  = tanh(einsum('bsl,sol->bso', h, W1) + b1)   [B, S, 3]
    h2  = tanh(einsum('bsk,sok->bso', h1, W2) + b2)  [B, S, 2]
    a, sb = h2[..., 0], h2[..., 1]
    out[b,i,j] = tanh(a[b,i]*W3[i,j,0] + sb[b,j]*W3[i,j,1] + b3[i,j])
    -> reshape [B, S*S]

Kernel strategy (pure data parallel over 8 cores, batch-sharded), fp16
end-to-end (measured rel err ~5e-4 against the fp64 oracle, tolerance 2e-2):
  * gather + batchnorm + Linear1 fold into one dense matmul x @ Weff.T;
    x is padded to 512 features with a ones column at 407 so the biases
    ride along as ordinary weight rows.  x is transposed on the host, so
    no PE or xbar transposes on device at all.
  * the pairwise head out[b,(i,j)] = tanh(a_i w0_ij + sb_j w1_ij + b_ij)
    runs as two PSUM-accumulated matmuls per 512-col window:
      - a-part: K=100 one-hot rows (nonzero iff k==i),
      - sb-part: K=101 (100 servers, nonzero iff k==j, + a ones row
        carrying b3).
    fp16 tables shrink the pairwise read from 8.2 MB f32 to 4.1 MB.
  * output is written as fp16 (half the HBM write traffic) in one
    contiguous 2.56 MB DMA per 128-row block, alternating sync/gpsimd
    rings; the host upcasts to fp32.
"""

import sys

import numpy as np

if "/opt/trn_rl_repo" not in sys.path:
    sys.path.insert(0, "/opt/trn_rl_repo")

import concourse.bacc as bacc
import concourse.mybir as mybir
from concourse import bass_utils
from concourse.tile import TileContext

S = 100
L = 13
FEAT = 4 * S + 7  # 407
FP = 512  # padded feature width (col 407 = ones, 408.. = zero)
B = 8192
EPS = 1e-5
N_CORES = 8
BL = B // N_CORES  # 1024 batch rows per core
SS = S * S  # 10000
WIN = 512  # output column window (one PSUM bank)
NWIN = (SS + WIN - 1) // WIN  # 20 (19 full + 272 tail)
F16 = mybir.dt.float16
F32 = mybir.dt.float32

# wt pack layout (columns of the [128, 1800] fp16 tile)
WT_WEFF = [0, 300, 600, 900]  # WeffT feature-chunk k at col 300k, width 300
WT_W2 = [1200, 1400, 1600]  # W2effT k-chunk c, width 200
WT_COLS = 1800

_module_cache = None


def _build_indices():
    idx = [[2 * i, 2 * i + 1] for i in range(S)]
    start = 2 * S
    for k in range(S):
        u, v = k, (k + 1) % S
        idx[u].extend([start, start + 1])
        idx[v].extend([start, start + 1])
        start += 2
    g0 = 4 * S
    for i in range(S):
        idx[i].extend(range(g0, g0 + 7))
    return np.asarray(idx, dtype=np.int64)


def _host_weights(inputs):
    f64 = np.float64
    gamma = np.asarray(inputs["gamma"], f64)
    beta = np.asarray(inputs["beta"], f64)
    mean = np.asarray(inputs["mean"], f64)
    var = np.asarray(inputs["var"], f64)
    W1 = np.asarray(inputs["W1"], f64)  # [S, 3, L]
    b1 = np.asarray(inputs["b1"], f64)  # [S, 3]
    W2 = np.asarray(inputs["W2"], f64)  # [S, 2, 3]
    b2 = np.asarray(inputs["b2"], f64)  # [S, 2]
    W3 = np.asarray(inputs["W3"], f64)  # [S, S, 2]
    b3 = np.asarray(inputs["b3"], f64)  # [S, S]
    idx = np.asarray(inputs["idx"], np.int64)  # [S, L]

    scale = gamma / np.sqrt(var + EPS)  # [S, L]
    shift = beta - mean * scale  # [S, L]

    # Weff[(s,o), f] = sum_l [idx[s,l]==f] W1[s,o,l]*scale[s,l]
    Wsc = W1 * scale[:, None, :]  # [S, 3, L]
    Weff = np.zeros((S, 3, FEAT), f64)
    s_ix = np.repeat(np.arange(S), 3 * L)
    o_ix = np.tile(np.repeat(np.arange(3), L), S)
    f_ix = np.repeat(idx[:, None, :], 3, axis=1).ravel()
    np.add.at(Weff, (s_ix, o_ix, f_ix), Wsc.ravel())
    Weff = Weff.reshape(3 * S, FEAT)
    beff = (b1 + np.einsum("sol,sl->so", W1, shift)).reshape(3 * S)

    # W2eff[(o2*S+s), (s*3+k)] = W2[s, o2, k]
    W2eff = np.zeros((2 * S, 3 * S), f64)
    for s in range(S):
        for o2 in range(2):
            W2eff[o2 * S + s, s * 3 : s * 3 + 3] = W2[s, o2, :]
    b2eff = b2.T.reshape(2 * S)

    wt = np.zeros((128, WT_COLS), np.float16)
    WeffT = Weff.T  # [FEAT, 300]
    for k, c in enumerate(WT_WEFF):
        f0 = 128 * k
        fw = min(128, FEAT - f0)
        wt[0:fw, c : c + 300] = WeffT[f0 : f0 + fw, :]
    wt[407 - 384, WT_WEFF[3] : WT_WEFF[3] + 300] = beff  # ones col at x[:,407]
    W2effT = W2eff.T  # [300, 200]
    for c_i, c in enumerate(WT_W2):
        wt[0:100, c : c + 200] = W2effT[c_i * 100 : (c_i + 1) * 100, :]
    wt[100, WT_W2[0] : WT_W2[0] + 200] = b2eff  # ones row of h1_0 carries b2

    # mt0[i, i*S+j] = W3[i,j,0] (one-hot in i)
    # mt1 rows 0..99: [j, i*S+j] = W3[i,j,1]; row 100 = b3 (vs asb ones row)
    mt0 = np.zeros((S, SS), np.float16)
    mt1 = np.zeros((S + 1, SS), np.float16)
    g = np.arange(SS)
    i_g = g // S
    j_g = g % S
    mt0[i_g, g] = W3[i_g, j_g, 0].astype(np.float16)
    mt1[j_g, g] = W3[i_g, j_g, 1].astype(np.float16)
    mt1[S, :] = b3.ravel()

    return {
        "wt": wt,
        "mt0": mt0,
        "mt1": mt1,
        "onesd": np.ones((1, 2048), np.float16),
    }


def _build_module():
    global _module_cache
    if _module_cache is not None:
        return _module_cache

    nc = bacc.Bacc("TRN2", target_bir_lowering=False, debug=False, num_devices=N_CORES)
    xin = nc.dram_tensor("xin", [FP, BL], F16, kind="ExternalInput").ap()
    wt_d = nc.dram_tensor("wt", [128, WT_COLS], F16, kind="ExternalInput").ap()
    mt0_d = nc.dram_tensor("mt0", [S, SS], F16, kind="ExternalInput").ap()
    mt1_d = nc.dram_tensor("mt1", [S + 1, SS], F16, kind="ExternalInput").ap()
    onesd = nc.dram_tensor("onesd", [1, 2048], F16, kind="ExternalInput").ap()
    yout = nc.dram_tensor("yout", [BL, SS], F16, kind="ExternalOutput").ap()

    TANH = mybir.ActivationFunctionType.Tanh

    with TileContext(nc) as tc:
        with (
            tc.tile_pool(name="const", bufs=1) as const,
            tc.tile_pool(name="ot_pool", bufs=2) as ot_pool,
            tc.tile_pool(name="ps_pool", bufs=2, space="PSUM") as ps_pool,
        ):
            # ---- loads ----
            # All bulk loads go through gpsimd (SWDGE): its descriptor
            # generation spreads partition lines across all 16 SDMA engines,
            # while HWDGE-dynamic loads clump onto one engine (~27 GiB/s).
            wt_t = const.tile([128, WT_COLS], F16)
            nc.gpsimd.dma_start(wt_t[:], wt_d[:, :])
            xT = []
            for k in range(4):
                xt = const.tile([128, BL], F16, name=f"xT_{k}", tag=f"xT{k}")
                nc.gpsimd.dma_start(xt[:], xin[128 * k : 128 * (k + 1), :])
                xT.append(xt)

            h1 = []
            for m in range(3):
                rows = 101 if m == 0 else 100
                t = const.tile([rows, BL], F16, name=f"h1_{m}", tag=f"h1{m}")
                h1.append(t)
            asb = const.tile([S + 1, 2 * BL], F16)
            nc.gpsimd.dma_start(h1[0][100:101, :], onesd[0:1, 0:BL])
            nc.gpsimd.dma_start(asb[S : S + 1, :], onesd[0:1, 0 : 2 * BL])

            mt0_t = const.tile([S, SS], F16)
            mt1_t = const.tile([S + 1, SS], F16)
            for e0 in range(0, SS, SS // 2):
                nc.gpsimd.dma_start(
                    mt0_t[:, e0 : e0 + SS // 2], mt0_d[:, e0 : e0 + SS // 2]
                )
                nc.gpsimd.dma_start(
                    mt1_t[:, e0 : e0 + SS // 2], mt1_d[:, e0 : e0 + SS // 2]
                )

            warm = const.tile([1, 8], F32)
            nc.scalar.activation(warm[:], wt_t[0:1, 0:8], TANH)  # tanh table preload

            # ---- front: h1 = tanh(x @ Weff.T + beff) ----
            for m in range(3):
                pm = ps_pool.tile([128, 2048], F32, name="pm", tag="ps")
                for h in range(2):
                    dst = pm[0:100, h * 512 : (h + 1) * 512]
                    for k in range(4):
                        nc.tensor.matmul(
                            dst,
                            wt_t[0:128, WT_WEFF[k] + 100 * m : WT_WEFF[k] + 100 * (m + 1)],
                            xT[k][0:128, h * 512 : (h + 1) * 512],
                            start=(k == 0),
                            stop=(k == 3),
                        )
                nc.scalar.activation(h1[m][0:100, :], pm[0:100, 0:BL], TANH)

            # ---- front: a/sb = tanh(h1 @ W2eff.T + b2eff) ----
            for g in range(2):
                pm2 = ps_pool.tile([128, 2048], F32, name="pm2", tag="ps")
                for h in range(2):
                    dst = pm2[0:100, h * 512 : (h + 1) * 512]
                    for c in range(3):
                        kr = 101 if c == 0 else 100
                        nc.tensor.matmul(
                            dst,
                            wt_t[0:kr, WT_W2[c] + 100 * g : WT_W2[c] + 100 * (g + 1)],
                            h1[c][0:kr, h * 512 : (h + 1) * 512],
                            start=(c == 0),
                            stop=(c == 2),
                        )
                nc.scalar.activation(
                    asb[0:S, g * BL : (g + 1) * BL], pm2[0:100, 0:BL], TANH
                )

            # ---- final: out = tanh(a_i w0 + sb_j w1 + b3) ----
            for bs in range(BL // 128):
                ot = ot_pool.tile([128, SS], F16, name="ot", tag="ot")
                ca = bs * 128
                cb = BL + bs * 128
                for q in range(5):
                    qw = min(2048, SS - q * 2048)
                    pf = ps_pool.tile([128, 2048], F32, name="pf", tag="ps")
                    for wl in range(4):
                        w = q * 4 + wl
                        c0 = WIN * w
                        cw = min(WIN, SS - c0)
                        off = 512 * wl
                        nc.tensor.matmul(
                            pf[0:128, off : off + cw],
                            asb[0:S, ca : ca + 128],
                            mt0_t[0:S, c0 : c0 + cw],
                            start=True,
                            stop=False,
                        )
                        nc.tensor.matmul(
                            pf[0:128, off : off + cw],
                            asb[0 : S + 1, cb : cb + 128],
                            mt1_t[0 : S + 1, c0 : c0 + cw],
                            start=False,
                            stop=True,
                        )
                    nc.scalar.activation(
                        ot[:, q * 2048 : q * 2048 + qw], pf[0:128, 0:qw], TANH
                    )
                    if q == 2:  # first half of the block is tanh'd — drain it
                        nc.sync.dma_start(
                            yout[bs * 128 : (bs + 1) * 128, 0 : SS // 2],
                            ot[:, 0 : SS // 2],
                        )
                nc.scalar.dma_start(
                    yout[bs * 128 : (bs + 1) * 128, SS // 2 : SS],
                    ot[:, SS // 2 : SS],
                )

    nc.compile()
    _module_cache = nc
    return nc


def _run(inputs, trace=False, trace_cores=None):
    nc = _build_module()
    hw = _host_weights(inputs)
    x = np.asarray(inputs["x"], np.float32)
    xpad = np.zeros((B, FP), np.float16)
    xpad[:, :FEAT] = x.astype(np.float16)
    xpad[:, FEAT] = 1.0
    in_maps = []
    for c in range(N_CORES):
        m = dict(hw)
        m["xin"] = np.ascontiguousarray(xpad[c * BL : (c + 1) * BL].T)
        in_maps.append(m)
    kwargs = {}
    if trace:
        bass_utils.upload_artifacts = lambda tmpdir: tmpdir  # no cloud store here
        kwargs = dict(trace=True, trace_cores=trace_cores or [0])
    res = bass_utils.run_bass_kernel_spmd(
        nc, in_maps, core_ids=list(range(N_CORES)), **kwargs
    )
    out = np.concatenate(
        [res.results[c]["yout"] for c in range(N_CORES)], axis=0
    ).astype(np.float32)
    return out, res


def kernel(**inputs) -> np.ndarray:
    out, _ = _run(inputs)
    return out


# revision 18
# speedup vs baseline: 1.7489x; 1.2969x over previous
"""Trainium2 Bass kernel for nn_DQN_57904749085018 (gnn_message_passing).

Computation (reference semantics):
    g   = x[:, idx]                                  [B, S, L] gather
    h   = (g - mean) * rsqrt(var+eps) * gamma + beta [B, S, L] batchnorm (eval)
    h1  = tanh(einsum('bsl,sol->bso', h, W1) + b1)   [B, S, 3]
    h2  = tanh(einsum('bsk,sok->bso', h1, W2) + b2)  [B, S, 2]
    a, sb = h2[..., 0], h2[..., 1]
    out[b,i,j] = tanh(a[b,i]*W3[i,j,0] + sb[b,j]*W3[i,j,1] + b3[i,j])
    -> reshape [B, S*S]

Kernel strategy (pure data parallel over 8 cores, batch-sharded), fp16
end-to-end (measured rel err ~5e-4 against the fp64 oracle, tolerance 2e-2):
  * gather + batchnorm + Linear1 fold into one dense matmul x @ Weff.T;
    x is padded to 512 features with a ones column at 407 so the biases
    ride along as ordinary weight rows.  x is transposed on the host, so
    no PE or xbar transposes on device at all.
  * the pairwise head out[b,(i,j)] = tanh(a_i w0_ij + sb_j w1_ij + b_ij)
    runs as two PSUM-accumulated matmuls per 512-col window:
      - a-part: K=100 one-hot rows (nonzero iff k==i),
      - sb-part: K=101 (100 servers, nonzero iff k==j, + a ones row
        carrying b3).
    fp16 tables shrink the pairwise read from 8.2 MB f32 to 4.1 MB.
  * output is written as fp16 (half the HBM write traffic) in one
    contiguous 2.56 MB DMA per 128-row block, alternating sync/gpsimd
    rings; the host upcasts to fp32.
"""

import sys

import numpy as np

if "/opt/trn_rl_repo" not in sys.path:
    sys.path.insert(0, "/opt/trn_rl_repo")

import concourse.bacc as bacc
import concourse.mybir as mybir
from concourse import bass_utils
from concourse.tile import TileContext

S = 100
L = 13
FEAT = 4 * S + 7  # 407
FP = 512  # padded feature width (col 407 = ones, 408.. = zero)
B = 8192
EPS = 1e-5
N_CORES = 8
BL = B // N_CORES  # 1024 batch rows per core
SS = S * S  # 10000
WIN = 512  # output column window (one PSUM bank)
NWIN = (SS + WIN - 1) // WIN  # 20 (19 full + 272 tail)
F16 = mybir.dt.float16
F32 = mybir.dt.float32

# wt pack layout (columns of the [128, 1800] fp16 tile)
WT_WEFF = [0, 300, 600, 900]  # WeffT feature-chunk k at col 300k, width 300
WT_W2 = [1200, 1400, 1600]  # W2effT k-chunk c, width 200
WT_COLS = 1800

_module_cache = None


def _build_indices():
    idx = [[2 * i, 2 * i + 1] for i in range(S)]
    start = 2 * S
    for k in range(S):
        u, v = k, (k + 1) % S
        idx[u].extend([start, start + 1])
        idx[v].extend([start, start + 1])
        start += 2
    g0 = 4 * S
    for i in range(S):
        idx[i].extend(range(g0, g0 + 7))
    return np.asarray(idx, dtype=np.int64)


def _host_weights(inputs):
    f64 = np.float64
    gamma = np.asarray(inputs["gamma"], f64)
    beta = np.asarray(inputs["beta"], f64)
    mean = np.asarray(inputs["mean"], f64)
    var = np.asarray(inputs["var"], f64)
    W1 = np.asarray(inputs["W1"], f64)  # [S, 3, L]
    b1 = np.asarray(inputs["b1"], f64)  # [S, 3]
    W2 = np.asarray(inputs["W2"], f64)  # [S, 2, 3]
    b2 = np.asarray(inputs["b2"], f64)  # [S, 2]
    W3 = np.asarray(inputs["W3"], f64)  # [S, S, 2]
    b3 = np.asarray(inputs["b3"], f64)  # [S, S]
    idx = np.asarray(inputs["idx"], np.int64)  # [S, L]

    scale = gamma / np.sqrt(var + EPS)  # [S, L]
    shift = beta - mean * scale  # [S, L]

    # Weff[(s,o), f] = sum_l [idx[s,l]==f] W1[s,o,l]*scale[s,l]
    Wsc = W1 * scale[:, None, :]  # [S, 3, L]
    Weff = np.zeros((S, 3, FEAT), f64)
    s_ix = np.repeat(np.arange(S), 3 * L)
    o_ix = np.tile(np.repeat(np.arange(3), L), S)
    f_ix = np.repeat(idx[:, None, :], 3, axis=1).ravel()
    np.add.at(Weff, (s_ix, o_ix, f_ix), Wsc.ravel())
    Weff = Weff.reshape(3 * S, FEAT)
    beff = (b1 + np.einsum("sol,sl->so", W1, shift)).reshape(3 * S)

    # W2eff[(o2*S+s), (s*3+k)] = W2[s, o2, k]
    W2eff = np.zeros((2 * S, 3 * S), f64)
    for s in range(S):
        for o2 in range(2):
            W2eff[o2 * S + s, s * 3 : s * 3 + 3] = W2[s, o2, :]
    b2eff = b2.T.reshape(2 * S)

    wt = np.zeros((128, WT_COLS), np.float16)
    WeffT = Weff.T  # [FEAT, 300]
    for k, c in enumerate(WT_WEFF):
        f0 = 128 * k
        fw = min(128, FEAT - f0)
        wt[0:fw, c : c + 300] = WeffT[f0 : f0 + fw, :]
    wt[407 - 384, WT_WEFF[3] : WT_WEFF[3] + 300] = beff  # ones col at x[:,407]
    W2effT = W2eff.T  # [300, 200]
    for c_i, c in enumerate(WT_W2):
        wt[0:100, c : c + 200] = W2effT[c_i * 100 : (c_i + 1) * 100, :]
    wt[100, WT_W2[0] : WT_W2[0] + 200] = b2eff  # ones row of h1_0 carries b2

    # mt0[i, i*S+j] = W3[i,j,0] (one-hot in i)
    # mt1 rows 0..99: [j, i*S+j] = W3[i,j,1]; row 100 = b3 (vs asb ones row)
    # Both padded to 128 partitions: full-partition DMAs spread across all
    # 16 SDMA engines, partial-partition ones clump onto one or two.
    mt0 = np.zeros((128, SS), np.float16)
    mt1 = np.zeros((128, SS), np.float16)
    g = np.arange(SS)
    i_g = g // S
    j_g = g % S
    mt0[i_g, g] = W3[i_g, j_g, 0].astype(np.float16)
    mt1[j_g, g] = W3[i_g, j_g, 1].astype(np.float16)
    mt1[S, :] = b3.ravel()

    return {
        "wt": wt,
        "mt0": mt0,
        "mt1": mt1,
        "onesd": np.ones((1, 2048), np.float16),
    }


def _build_module():
    global _module_cache
    if _module_cache is not None:
        return _module_cache

    nc = bacc.Bacc("TRN2", target_bir_lowering=False, debug=False, num_devices=N_CORES)
    xin = nc.dram_tensor("xin", [FP, BL], F16, kind="ExternalInput").ap()
    wt_d = nc.dram_tensor("wt", [128, WT_COLS], F16, kind="ExternalInput").ap()
    mt0_d = nc.dram_tensor("mt0", [128, SS], F16, kind="ExternalInput").ap()
    mt1_d = nc.dram_tensor("mt1", [128, SS], F16, kind="ExternalInput").ap()
    onesd = nc.dram_tensor("onesd", [1, 2048], F16, kind="ExternalInput").ap()
    yout = nc.dram_tensor("yout", [BL, SS], F16, kind="ExternalOutput").ap()

    TANH = mybir.ActivationFunctionType.Tanh

    with TileContext(nc) as tc:
        with (
            tc.tile_pool(name="const", bufs=1) as const,
            tc.tile_pool(name="ot_pool", bufs=2) as ot_pool,
            tc.tile_pool(name="ps_pool", bufs=2, space="PSUM") as ps_pool,
        ):
            # ---- loads ----
            # All bulk loads go through gpsimd (SWDGE): its descriptor
            # generation spreads partition lines across all 16 SDMA engines,
            # while HWDGE-dynamic loads clump onto one engine (~27 GiB/s).
            wt_t = const.tile([128, WT_COLS], F16)
            nc.gpsimd.dma_start(wt_t[:], wt_d[:, :])
            xT = []
            for k in range(4):
                xt = const.tile([128, BL], F16, name=f"xT_{k}", tag=f"xT{k}")
                nc.gpsimd.dma_start(xt[:], xin[128 * k : 128 * (k + 1), :])
                xT.append(xt)

            h1 = []
            for m in range(3):
                rows = 101 if m == 0 else 100
                t = const.tile([rows, BL], F16, name=f"h1_{m}", tag=f"h1{m}")
                h1.append(t)
            asb = const.tile([S + 1, 2 * BL], F16)
            nc.gpsimd.dma_start(h1[0][100:101, :], onesd[0:1, 0:BL])
            nc.gpsimd.dma_start(asb[S : S + 1, :], onesd[0:1, 0 : 2 * BL])

            mt0_t = const.tile([128, SS], F16)
            mt1_t = const.tile([128, SS], F16)
            for e0 in range(0, SS, SS // 2):
                nc.gpsimd.dma_start(
                    mt0_t[:, e0 : e0 + SS // 2], mt0_d[:, e0 : e0 + SS // 2]
                )
                nc.gpsimd.dma_start(
                    mt1_t[:, e0 : e0 + SS // 2], mt1_d[:, e0 : e0 + SS // 2]
                )

            warm = const.tile([1, 8], F32)
            nc.scalar.activation(warm[:], wt_t[0:1, 0:8], TANH)  # tanh table preload

            # ---- front: h1 = tanh(x @ Weff.T + beff) ----
            for m in range(3):
                pm = ps_pool.tile([128, 2048], F32, name="pm", tag="ps")
                for h in range(2):
                    dst = pm[0:100, h * 512 : (h + 1) * 512]
                    for k in range(4):
                        nc.tensor.matmul(
                            dst,
                            wt_t[0:128, WT_WEFF[k] + 100 * m : WT_WEFF[k] + 100 * (m + 1)],
                            xT[k][0:128, h * 512 : (h + 1) * 512],
                            start=(k == 0),
                            stop=(k == 3),
                        )
                nc.scalar.activation(h1[m][0:100, :], pm[0:100, 0:BL], TANH)

            # ---- front: a/sb = tanh(h1 @ W2eff.T + b2eff) ----
            for g in range(2):
                pm2 = ps_pool.tile([128, 2048], F32, name="pm2", tag="ps")
                for h in range(2):
                    dst = pm2[0:100, h * 512 : (h + 1) * 512]
                    for c in range(3):
                        kr = 101 if c == 0 else 100
                        nc.tensor.matmul(
                            dst,
                            wt_t[0:kr, WT_W2[c] + 100 * g : WT_W2[c] + 100 * (g + 1)],
                            h1[c][0:kr, h * 512 : (h + 1) * 512],
                            start=(c == 0),
                            stop=(c == 2),
                        )
                nc.scalar.activation(
                    asb[0:S, g * BL : (g + 1) * BL], pm2[0:100, 0:BL], TANH
                )

            # ---- final: out = tanh(a_i w0 + sb_j w1 + b3) ----
            for bs in range(BL // 128):
                ot = ot_pool.tile([128, SS], F16, name="ot", tag="ot")
                ca = bs * 128
                cb = BL + bs * 128
                for q in range(5):
                    qw = min(2048, SS - q * 2048)
                    pf = ps_pool.tile([128, 2048], F32, name="pf", tag="ps")
                    for wl in range(4):
                        w = q * 4 + wl
                        c0 = WIN * w
                        cw = min(WIN, SS - c0)
                        off = 512 * wl
                        nc.tensor.matmul(
                            pf[0:128, off : off + cw],
                            asb[0:S, ca : ca + 128],
                            mt0_t[0:S, c0 : c0 + cw],
                            start=True,
                            stop=False,
                        )
                        nc.tensor.matmul(
                            pf[0:128, off : off + cw],
                            asb[0 : S + 1, cb : cb + 128],
                            mt1_t[0 : S + 1, c0 : c0 + cw],
                            start=False,
                            stop=True,
                        )
                    nc.scalar.activation(
                        ot[:, q * 2048 : q * 2048 + qw], pf[0:128, 0:qw], TANH
                    )
                    if q == 2:  # first half of the block is tanh'd — drain it
                        nc.sync.dma_start(
                            yout[bs * 128 : (bs + 1) * 128, 0 : SS // 2],
                            ot[:, 0 : SS // 2],
                        )
                nc.scalar.dma_start(
                    yout[bs * 128 : (bs + 1) * 128, SS // 2 : SS],
                    ot[:, SS // 2 : SS],
                )

    nc.compile()
    _module_cache = nc
    return nc


def _run(inputs, trace=False, trace_cores=None):
    nc = _build_module()
    hw = _host_weights(inputs)
    x = np.asarray(inputs["x"], np.float32)
    xpad = np.zeros((B, FP), np.float16)
    xpad[:, :FEAT] = x.astype(np.float16)
    xpad[:, FEAT] = 1.0
    in_maps = []
    for c in range(N_CORES):
        m = dict(hw)
        m["xin"] = np.ascontiguousarray(xpad[c * BL : (c + 1) * BL].T)
        in_maps.append(m)
    kwargs = {}
    if trace:
        bass_utils.upload_artifacts = lambda tmpdir: tmpdir  # no cloud store here
        kwargs = dict(trace=True, trace_cores=trace_cores or [0])
    res = bass_utils.run_bass_kernel_spmd(
        nc, in_maps, core_ids=list(range(N_CORES)), **kwargs
    )
    out = np.concatenate(
        [res.results[c]["yout"] for c in range(N_CORES)], axis=0
    ).astype(np.float32)
    return out, res


def kernel(**inputs) -> np.ndarray:
    out, _ = _run(inputs)
    return out
